# revision 12
# baseline (speedup 1.0000x reference)
"""Trainium2 Bass kernel for the DreamerV3-style ActorCriticLoss.

Contract: kernel(**inputs) takes the FULL (unsharded) numpy inputs and
returns the FULL output (a float32 scalar loss). The batch dim (B=4096) is
sharded 8 ways (pure data parallel, 512 rows/core).

Device strategy (per core): inputs are staged host-side in a TRANSPOSED
[t, bin, row] fp16 layout so the 255-bin axis lands on SBUF partitions.
The device then only does: DMA in, exp (ACT), two elementwise products
(DVE), and bin-dim reductions as Tensor-engine matmuls against constant
(ones|bins) stationary vectors, accumulating every per-(row,t) statistic
into a single PSUM bank [128, 512]:
  parts  0..31 : (sum_r, wsum_r) interleaved per t   (reward softmax stats)
  parts 32..63 : (sum_s, wsum_s) interleaved per t   (slow-critic stats)
  parts 64..79 : sum_f per t                         (fast-critic lse denom)
  parts 80..95 : fdot per t                          (sum exp(slw)*fst)
  parts 96..111: sum_a per t                         (action softmax denom)
  parts112..127: padot per t                         (sum exp(act)*act)
One DVE copy ships the bank to SBUF; one DMA ships it to the host, which
finishes the cheap O(B*T) tail in numpy float64: symexp decodes, sigmoid,
lambda scan, quantiles, two-hot gather (from the original fp32 inputs),
log-sum-exp logs, and the final scalar assembly.

Self-contained: hardcodes shapes from the problem spec.
"""

import sys
from contextlib import ExitStack

sys.path.insert(0, "/opt/trn_rl_repo")

import numpy as np

import concourse.bass as bass  # noqa: E402
import concourse.bacc as bacc  # noqa: E402
import concourse.mybir as mybir  # noqa: E402
from concourse import bass_utils  # noqa: E402
from concourse import tile  # noqa: E402

# ---- problem constants (from reference.py) ----
LOW, HIGH, NBINS = -20.0, 20.0, 255
GAMMA, LAM = 0.99, 0.95
ENT_COEF, SLOW_W = 0.05, 1.0
STEP = (HIGH - LOW) / (NBINS - 1)
B, T, A = 4096, 16, 32

NCORES = 8
R = B // NCORES  # 512 rows per core
NP2 = 256  # padded bin count (2 chunks of 128)
G = 4  # timesteps per pipeline group
NG = T // G  # 4 groups
PAD_VAL = -10.37  # maps to ~0 under both ACT exp and the Schraudolph trick
EXP_A = 1477.0  # 1024*log2(e) for fp16 Schraudolph exp
EXP_B = 15316.0  # fp16 exponent bias magic, incl. -44 error-centering
LSE_F_BIAS = 0.01051  # systematic ln-sum bias of the Schraudolph exp (HW-calibrated)

F32 = mybir.dt.float32
F16 = mybir.dt.float16
F8 = mybir.dt.float8e4
I16 = mybir.dt.int16
Alu = mybir.AluOpType
Act = mybir.ActivationFunctionType


def build_kernel(nc: bass.Bass, tc: "tile.TileContext"):
    # ---- DRAM I/O (transposed, fp16, host-prepared) ----
    GW0 = G * 2 * R
    rew_d = nc.dram_tensor("rew", [NG, 128, GW0], F8, kind="ExternalInput").ap()
    slw_d = nc.dram_tensor("slw", [NG, 128, GW0], F8, kind="ExternalInput").ap()
    fst_d = nc.dram_tensor("fst", [NG, 128, GW0], F8, kind="ExternalInput").ap()
    actl_d = nc.dram_tensor("actl", [NG, 128, R], F8, kind="ExternalInput").ap()
    wmat_d = nc.dram_tensor("wmat", [128, 72 * 32], F16, kind="ExternalInput").ap()
    stats_d = nc.dram_tensor("stats", [128, R], F32, kind="ExternalOutput").ap()

    GW = G * 2 * R  # 4096 free elements per group tile

    ctx = ExitStack()
    const_pool = ctx.enter_context(tc.tile_pool(name="const", bufs=1))
    raw_pool = ctx.enter_context(tc.tile_pool(name="raw", bufs=2))
    e_pool = ctx.enter_context(tc.tile_pool(name="exps", bufs=2))
    prod_pool = ctx.enter_context(tc.tile_pool(name="prod", bufs=2))
    act_pool = ctx.enter_context(tc.tile_pool(name="act", bufs=2))
    psum_pool = ctx.enter_context(tc.psum_pool(name="psum", bufs=1))
    out_pool = ctx.enter_context(tc.tile_pool(name="outp", bufs=1))

    wsb = const_pool.tile([128, 72 * 32], F16, name="wsb", tag="wsb")
    wsb3 = wsb[:].rearrange("p (w k) -> p w k", k=32)

    psum_a = psum_pool.tile([64, R], F32, name="psum_a", tag="psum_a")
    psum_b = psum_pool.tile([64, R], F32, name="psum_b", tag="psum_b")
    qtile = [psum_a, psum_a, psum_b, psum_b]
    qoff = [0, 32, 0, 32]
    qfirst = [True, True, True, True]  # start flag per PSUM quadrant

    def mm(q, w_idx, rhs, last=False):
        o = qoff[q]
        nc.tensor.matmul(
            qtile[q][o:o + 32, :], wsb3[:, w_idx, :], rhs,
            start=qfirst[q], stop=last, skip_group_check=True,
        )
        qfirst[q] = False

    for g in range(NG):
        raw_s = raw_pool.tile([128, GW], F8, name=f"raw_s{g}", tag="raw_s")
        nc.sync.dma_start(out=raw_s[:], in_=slw_d[g])
        raw_r = raw_pool.tile([128, GW], F8, name=f"raw_r{g}", tag="raw_r")
        nc.sync.dma_start(out=raw_r[:], in_=rew_d[g])
        raw_f = raw_pool.tile([128, GW], F8, name=f"raw_f{g}", tag="raw_f")
        nc.sync.dma_start(out=raw_f[:], in_=fst_d[g])
        acl = act_pool.tile([128, R], F8, name=f"acl{g}", tag="acl")
        nc.sync.dma_start(out=acl[:], in_=actl_d[g])
        if g == 0:
            nc.sync.dma_start(out=wsb[:], in_=wmat_d)

        HW2 = GW // 2
        e_s = e_pool.tile([128, GW], F16, name=f"e_s{g}", tag="e_s")
        nc.scalar.activation(e_s[:], raw_s[:], Act.Exp)
        e_r = e_pool.tile([128, GW], F16, name=f"e_r{g}", tag="e_r")
        nc.scalar.activation(e_r[:], raw_r[:], Act.Exp)

        # fst upcast once (fp8 operands would break DVE fast modes), then
        # e_f = exp(fst) via the Schraudolph bit-trick: entirely on the DVE
        f16f = prod_pool.tile([128, GW], F16, name=f"f16f{g}", tag="f16f")
        nc.vector.tensor_copy(f16f[:], raw_f[:])
        t16 = prod_pool.tile([128, GW], F16, name=f"t16_{g}", tag="t16")
        nc.vector.tensor_scalar(t16[:], f16f[:], EXP_A, EXP_B, Alu.mult, Alu.add)
        i16 = prod_pool.tile([128, GW], I16, name=f"i16_{g}", tag="i16")
        nc.vector.tensor_copy(i16[:], t16[:])
        e_f = i16[:].bitcast(F16)

        # e_a likewise on the DVE (lse_a bias corrected on host)
        acl16 = act_pool.tile([128, R], F16, name=f"acl16{g}", tag="acl16")
        nc.vector.tensor_copy(acl16[:], acl[:])
        t16a = act_pool.tile([128, R], F16, name=f"t16a{g}", tag="t16a")
        nc.vector.tensor_scalar(t16a[:], acl16[:], EXP_A, EXP_B, Alu.mult, Alu.add)
        i16a = act_pool.tile([128, R], I16, name=f"i16a{g}", tag="i16a")
        nc.vector.tensor_copy(i16a[:], t16a[:])
        e_a = i16a[:].bitcast(F16)

        # fdot product: half on DVE, half on the otherwise-idle GpSimd
        prod = prod_pool.tile([128, GW], F16, name=f"prod{g}", tag="prod")
        nc.vector.tensor_mul(prod[:, :HW2], e_s[:, :HW2], f16f[:, :HW2])
        with nc.allow_low_precision("fp16 product"):
            nc.gpsimd.tensor_mul(prod[:, HW2:], e_s[:, HW2:], f16f[:, HW2:])
        prod_a = act_pool.tile([128, R], F16, name=f"prod_a{g}", tag="prod_a")
        nc.vector.tensor_mul(prod_a[:], e_a, acl16[:])

        last_g = g == NG - 1
        for j in range(G):
            t = G * g + j
            last_t = last_g and j == G - 1
            for c in range(2):
                sl = slice((j * 2 + c) * R, (j * 2 + c + 1) * R)
                last = last_t and c == 1
                mm(0, 2 * t + c, e_r[:, sl], last)
                mm(1, 2 * t + c, e_s[:, sl], last)
                mm(2, 32 + 2 * t, e_f[:, sl], last and False)  # noqa
                mm(2, 32 + 2 * t + 1, prod[:, sl], last)

        mm(3, 64 + g, e_a, False)
        mm(3, 68 + g, prod_a[:], last_g)

    stats = out_pool.tile([128, R], F32, name="stats", tag="stats")
    nc.vector.tensor_copy(stats[0:64, :], psum_a[:])
    nc.vector.tensor_copy(stats[64:128, :], psum_b[:])
    nc.sync.dma_start(out=stats_d, in_=stats[:])

    ctx.close()


def _install_ntff_hook_shim():
    """This image's `antenv` lacks `axon_hooks`; replicate the boot-time
    NTFF profile hook (ctypes into libaxon_pjrt.so) so trace=True works."""
    try:
        from antenv.axon_hooks import get_axon_ntff_profile_hook  # noqa: F401

        return
    except ImportError:
        pass
    import contextlib
    import ctypes
    import types

    so_path = "/opt/axon/libaxon_pjrt.so"
    hook = None
    try:
        lib = ctypes.CDLL(so_path)
        if hasattr(lib, "axon_start_nrt_profile"):
            lib.axon_start_nrt_profile.argtypes = [
                ctypes.POINTER(ctypes.c_int64),
                ctypes.c_size_t,
            ]
            lib.axon_start_nrt_profile.restype = ctypes.c_int64
            lib.axon_stop_nrt_profile.argtypes = [ctypes.c_char_p]
            lib.axon_stop_nrt_profile.restype = ctypes.c_int64

            @contextlib.contextmanager
            def _hook(output_dir, device_ids):
                import jax

                jax.devices()
                if device_ids:
                    ids = (ctypes.c_int64 * len(device_ids))(*device_ids)
                    rc = lib.axon_start_nrt_profile(ids, len(device_ids))
                else:
                    rc = lib.axon_start_nrt_profile(None, 0)
                if rc != 0:
                    raise RuntimeError(f"axon_start_nrt_profile rc={rc}")
                try:
                    yield
                finally:
                    n = lib.axon_stop_nrt_profile(str(output_dir).encode())
                    if n < 0:
                        raise RuntimeError(f"axon_stop_nrt_profile rc={n}")
                    print(f"profile: {n} file(s) written to {output_dir}")

            hook = _hook
    except OSError:
        pass

    mod = types.ModuleType("antenv.axon_hooks")
    mod._hook = hook
    mod.get_axon_ntff_profile_hook = lambda: mod._hook
    mod.set_axon_ntff_profile_hook = lambda h: setattr(mod, "_hook", h)
    sys.modules["antenv.axon_hooks"] = mod


_CACHE = {}


def _get_compiled():
    if "nc" not in _CACHE:
        nc = bacc.Bacc(
            "TRN2", target_bir_lowering=False, debug=False, num_devices=NCORES
        )
        with tile.TileContext(nc) as tc:
            build_kernel(nc, tc)
        nc.compile()
        _CACHE["nc"] = nc
    return _CACHE["nc"]


def _wmat():
    bins = (np.arange(NBINS) * STEP + LOW).astype(np.float32)
    binc = np.zeros((2, 128), np.float32)
    binc[0] = bins[:128]
    binc[1, :127] = bins[128:]
    wm = np.zeros((72, 128, 32), np.float16)
    for t in range(T):
        for c in range(2):
            wm[2 * t + c, :, 2 * t % 32] = 1.0
            wm[2 * t + c, :, (2 * t + 1) % 32] = binc[c]
    for k in range(32):
        wm[32 + k, :, k] = 1.0
    for g in range(NG):
        for j in range(G):
            t = G * g + j
            wm[64 + g, 32 * j:32 * j + 32, t % 32] = 1.0
            wm[68 + g, 32 * j:32 * j + 32, (16 + t) % 32] = 1.0
    return np.ascontiguousarray(wm.transpose(1, 0, 2)).reshape(128, 72 * 32)


from ml_dtypes import float8_e4m3fn as _f8  # noqa: E402


def _stage_bins_tensor(x, dt):
    """[B, T, NBINS] fp32 -> per-core flat [NG, 128, G*2*R] so each
    SBUF partition line is one contiguous run: [g, p, (j c r)] =
    x[row=r, t=4g+j, bin=c*128+p]."""
    x8 = x.astype(dt)  # cast first: transpose then moves fewer bytes
    out = np.full((NCORES, T, NP2, R), dt(PAD_VAL), dt)
    # [core, row, t, n] -> [core, t, n, row]
    out[:, :, :NBINS, :] = x8.reshape(NCORES, R, T, NBINS).transpose(0, 2, 3, 1)
    # [core, (g j), (c p), r] -> [core, g, p, (j c r)]
    out = np.ascontiguousarray(
        out.reshape(NCORES, NG, G, 2, 128, R).transpose(0, 1, 4, 2, 3, 5)
    ).reshape(NCORES, NG, 128, G * 2 * R)
    return out


def _make_in_maps(inputs):
    rew = _stage_bins_tensor(np.asarray(inputs["predicted_reward_logits"]), _f8)
    slw = _stage_bins_tensor(np.asarray(inputs["slow_critic_logits"]), _f8)
    fst = _stage_bins_tensor(np.asarray(inputs["fast_critic_logits"]), _f8)
    actl = np.asarray(inputs["action_logits"]).astype(_f8)
    # [core, row, (g j), a] -> [core, g, (j a), row]
    actl_t = np.ascontiguousarray(
        actl.reshape(NCORES, R, NG, G, A).transpose(0, 2, 3, 4, 1)
    ).reshape(NCORES, NG, G * A, R)
    wm = _wmat()
    return [
        {
            "rew": rew[i],
            "slw": slw[i],
            "fst": fst[i],
            "actl": actl_t[i],
            "wmat": wm,
        }
        for i in range(NCORES)
    ]


def _combine(inputs, results):
    """Host tail in float64: decode stats, lambda scan, quantiles, two-hot
    gather, final scalar."""
    S = np.stack([np.asarray(r["stats"], dtype=np.float64) for r in results])
    # S: [core, 128, R]; reassemble [B, T] quantities (row = core*R + r)
    idx_t = np.arange(T)

    def grab(base, stride=1, off=0):
        # partitions base + stride*t (+off), -> [B, T]
        parts = S[:, base + off + stride * idx_t, :]  # [core, T, R]
        return parts.transpose(0, 2, 1).reshape(B, T)

    sum_r = grab(0, 2)
    wsum_r = grab(0, 2, 1)
    sum_s = grab(32, 2)
    wsum_s = grab(32, 2, 1)
    sum_f = grab(64, 2)
    fdot = grab(64, 2, 1)
    sum_a = grab(96)
    padot = grab(112)

    def symexp(y):
        return np.sign(y) * (np.exp(np.abs(y)) - 1.0)

    rewards = symexp(wsum_r / sum_r)
    values = symexp(wsum_s / sum_s)
    cont = np.asarray(
        inputs["predicted_continue_logits"], dtype=np.float64
    )[..., 0]
    continues = 1.0 / (1.0 + np.exp(-cont))

    # lambda returns (vectorized over B, reverse scan over T)
    lam_ret = np.empty((B, T), np.float64)
    lam_ret[:, -1] = values[:, -1]
    for t in range(T - 2, -1, -1):
        lam_ret[:, t] = rewards[:, t] + GAMMA * continues[:, t] * (
            (1.0 - LAM) * values[:, t + 1] + LAM * lam_ret[:, t + 1]
        )

    # ---- actor ----
    actl = np.asarray(inputs["action_logits"], dtype=np.float64)
    actions = np.asarray(inputs["actions"]).astype(np.int64)
    alp_raw = np.take_along_axis(actl, actions[..., None], axis=-1)[..., 0]
    lse_a = np.log(sum_a) - LSE_F_BIAS
    alp = alp_raw - lse_a
    ent = lse_a - padot / sum_a
    flat = lam_ret.reshape(-1)
    p_hi = np.quantile(flat, 0.95)
    p_lo = np.quantile(flat, 0.05)
    norm = max(p_hi - p_lo, 1.0)
    norm_adv = (lam_ret - values) / norm
    actor = -np.mean(norm_adv * alp) - ENT_COEF * np.mean(ent)

    # ---- critic ----
    y2 = np.sign(lam_ret) * np.log1p(np.abs(lam_ret))
    pos = (np.clip(y2, LOW, HIGH) - LOW) / STEP
    k = np.clip(np.floor(pos), 0, NBINS - 2).astype(np.int64)
    w = pos - k
    fst = np.asarray(inputs["fast_critic_logits"], dtype=np.float64)
    fk = np.take_along_axis(fst, k[..., None], axis=-1)[..., 0]
    fk1 = np.take_along_axis(fst, k[..., None] + 1, axis=-1)[..., 0]
    g = (1.0 - w) * fk + w * fk1
    lse_f = np.log(sum_f) - LSE_F_BIAS
    fdn = fdot / sum_s
    critic = np.mean(lse_f - g) + SLOW_W * np.mean(lse_f - fdn)

    return np.float32(actor + critic)


def run(inputs, trace=False, **kw):
    if trace:
        _install_ntff_hook_shim()
    nc = _get_compiled()
    in_maps = _make_in_maps(inputs)
    res = bass_utils.run_bass_kernel_spmd(
        nc, in_maps, core_ids=list(range(NCORES)), trace=trace, **kw
    )
    return _combine(inputs, res.results), res


def kernel(**inputs) -> np.ndarray:
    out, _ = run(inputs)
    return out


# revision 13
# speedup vs baseline: 1.2404x; 1.2404x over previous
"""Trainium2 Bass kernel for the DreamerV3-style ActorCriticLoss.

Contract: kernel(**inputs) takes the FULL (unsharded) numpy inputs and
returns the FULL output (a float32 scalar loss). The batch dim (B=4096) is
sharded 8 ways (pure data parallel, 512 rows/core).

Device strategy (per core): inputs are staged host-side in a TRANSPOSED
[t, bin, row] fp16 layout so the 255-bin axis lands on SBUF partitions.
The device then only does: DMA in, exp (ACT), two elementwise products
(DVE), and bin-dim reductions as Tensor-engine matmuls against constant
(ones|bins) stationary vectors, accumulating every per-(row,t) statistic
into a single PSUM bank [128, 512]:
  parts  0..31 : (sum_r, wsum_r) interleaved per t   (reward softmax stats)
  parts 32..63 : (sum_s, wsum_s) interleaved per t   (slow-critic stats)
  parts 64..79 : sum_f per t                         (fast-critic lse denom)
  parts 80..95 : fdot per t                          (sum exp(slw)*fst)
  parts 96..111: sum_a per t                         (action softmax denom)
  parts112..127: padot per t                         (sum exp(act)*act)
One DVE copy ships the bank to SBUF; one DMA ships it to the host, which
finishes the cheap O(B*T) tail in numpy float64: symexp decodes, sigmoid,
lambda scan, quantiles, two-hot gather (from the original fp32 inputs),
log-sum-exp logs, and the final scalar assembly.

Self-contained: hardcodes shapes from the problem spec.
"""

import sys
from contextlib import ExitStack

sys.path.insert(0, "/opt/trn_rl_repo")

import numpy as np

import concourse.bass as bass  # noqa: E402
import concourse.bacc as bacc  # noqa: E402
import concourse.mybir as mybir  # noqa: E402
from concourse import bass_utils  # noqa: E402
from concourse import tile  # noqa: E402

# ---- problem constants (from reference.py) ----
LOW, HIGH, NBINS = -20.0, 20.0, 255
GAMMA, LAM = 0.99, 0.95
ENT_COEF, SLOW_W = 0.05, 1.0
STEP = (HIGH - LOW) / (NBINS - 1)
B, T, A = 4096, 16, 32

NCORES = 8
R = B // NCORES  # 512 rows per core
NP2 = 256  # padded bin count (2 chunks of 128)
G = 4  # timesteps per pipeline group
NG = T // G  # 4 groups
PAD_VAL = -10.37  # maps to ~0 under both ACT exp and the Schraudolph trick
EXP_A = 1477.0  # 1024*log2(e) for fp16 Schraudolph exp
EXP_B = 15316.0  # fp16 exponent bias magic, incl. -44 error-centering
LSE_F_BIAS = 0.01051  # systematic ln-sum bias of the Schraudolph exp (HW-calibrated)

F32 = mybir.dt.float32
F16 = mybir.dt.float16
F8 = mybir.dt.float8e4
I16 = mybir.dt.int16
Alu = mybir.AluOpType
Act = mybir.ActivationFunctionType


def build_kernel(nc: bass.Bass, tc: "tile.TileContext"):
    # ---- DRAM I/O (transposed, fp16, host-prepared) ----
    GW0 = G * 2 * R
    rew_d = nc.dram_tensor("rew", [NG, 128, GW0], F8, kind="ExternalInput").ap()
    slw_d = nc.dram_tensor("slw", [NG, 128, GW0], F8, kind="ExternalInput").ap()
    fst_d = nc.dram_tensor("fst", [NG, 128, GW0], F8, kind="ExternalInput").ap()
    actl_d = nc.dram_tensor("actl", [NG, 128, R], F8, kind="ExternalInput").ap()
    wmat_d = nc.dram_tensor("wmat", [128, 72 * 32], F16, kind="ExternalInput").ap()
    stats_d = nc.dram_tensor("stats", [128, R], F32, kind="ExternalOutput").ap()

    GW = G * 2 * R  # 4096 free elements per group tile

    ctx = ExitStack()
    const_pool = ctx.enter_context(tc.tile_pool(name="const", bufs=1))
    raw_pool = ctx.enter_context(tc.tile_pool(name="raw", bufs=2))
    e_pool = ctx.enter_context(tc.tile_pool(name="exps", bufs=2))
    prod_pool = ctx.enter_context(tc.tile_pool(name="prod", bufs=2))
    act_pool = ctx.enter_context(tc.tile_pool(name="act", bufs=2))
    psum_pool = ctx.enter_context(tc.psum_pool(name="psum", bufs=1))
    out_pool = ctx.enter_context(tc.tile_pool(name="outp", bufs=1))

    wsb = const_pool.tile([128, 72 * 32], F16, name="wsb", tag="wsb")
    wsb3 = wsb[:].rearrange("p (w k) -> p w k", k=32)

    psum_a = psum_pool.tile([64, R], F32, name="psum_a", tag="psum_a")
    psum_b = psum_pool.tile([64, R], F32, name="psum_b", tag="psum_b")
    qtile = [psum_a, psum_a, psum_b, psum_b]
    qoff = [0, 32, 0, 32]
    qfirst = [True, True, True, True]  # start flag per PSUM quadrant

    def mm(q, w_idx, rhs, last=False):
        o = qoff[q]
        nc.tensor.matmul(
            qtile[q][o:o + 32, :], wsb3[:, w_idx, :], rhs,
            start=qfirst[q], stop=last, skip_group_check=True,
        )
        qfirst[q] = False

    for g in range(NG):
        raw_s = raw_pool.tile([128, GW], F8, name=f"raw_s{g}", tag="raw_s")
        if g == 0:
            nc.sync.dma_start(out=raw_s[:, :GW // 2], in_=slw_d[g][:, :GW // 2])
            nc.sync.dma_start(out=raw_s[:, GW // 2:], in_=slw_d[g][:, GW // 2:])
        else:
            nc.sync.dma_start(out=raw_s[:], in_=slw_d[g])
        raw_r = raw_pool.tile([128, GW], F8, name=f"raw_r{g}", tag="raw_r")
        nc.sync.dma_start(out=raw_r[:], in_=rew_d[g])
        raw_f = raw_pool.tile([128, GW], F8, name=f"raw_f{g}", tag="raw_f")
        nc.sync.dma_start(out=raw_f[:], in_=fst_d[g])
        acl = act_pool.tile([128, R], F8, name=f"acl{g}", tag="acl")
        nc.sync.dma_start(out=acl[:], in_=actl_d[g])
        if g == 0:
            nc.sync.dma_start(out=wsb[:], in_=wmat_d)

        HW2 = GW // 2
        e_s = e_pool.tile([128, GW], F16, name=f"e_s{g}", tag="e_s")
        if g == 0:
            nc.scalar.activation(e_s[:, :HW2], raw_s[:, :HW2], Act.Exp)
            nc.scalar.activation(e_s[:, HW2:], raw_s[:, HW2:], Act.Exp)
        else:
            nc.scalar.activation(e_s[:], raw_s[:], Act.Exp)
        e_r = e_pool.tile([128, GW], F16, name=f"e_r{g}", tag="e_r")
        nc.scalar.activation(e_r[:], raw_r[:], Act.Exp)

        # fst upcast once (fp8 operands would break DVE fast modes), then
        # e_f = exp(fst) via the Schraudolph bit-trick: entirely on the DVE
        f16f = prod_pool.tile([128, GW], F16, name=f"f16f{g}", tag="f16f")
        nc.vector.tensor_copy(f16f[:], raw_f[:])
        t16 = prod_pool.tile([128, GW], F16, name=f"t16_{g}", tag="t16")
        nc.vector.tensor_scalar(t16[:], f16f[:], EXP_A, EXP_B, Alu.mult, Alu.add)
        i16 = prod_pool.tile([128, GW], I16, name=f"i16_{g}", tag="i16")
        nc.vector.tensor_copy(i16[:], t16[:])
        e_f = i16[:].bitcast(F16)

        e_a = act_pool.tile([128, R], F16, name=f"e_a{g}", tag="e_a")
        nc.scalar.activation(e_a[:], acl[:], Act.Exp)

        prod = prod_pool.tile([128, GW], F16, name=f"prod{g}", tag="prod")
        nc.vector.tensor_mul(prod[:], e_s[:], f16f[:])
        prod_a = act_pool.tile([128, R], F16, name=f"prod_a{g}", tag="prod_a")
        nc.vector.tensor_mul(prod_a[:], e_a[:], acl[:])

        last_g = g == NG - 1
        for j in range(G):
            t = G * g + j
            last_t = last_g and j == G - 1
            for c in range(2):
                sl = slice((j * 2 + c) * R, (j * 2 + c + 1) * R)
                last = last_t and c == 1
                mm(0, 2 * t + c, e_r[:, sl], last)
                mm(1, 2 * t + c, e_s[:, sl], last)
                mm(2, 32 + 2 * t, e_f[:, sl], last and False)  # noqa
                mm(2, 32 + 2 * t + 1, prod[:, sl], last)

        mm(3, 64 + g, e_a[:], False)
        mm(3, 68 + g, prod_a[:], last_g)

    stats = out_pool.tile([128, R], F32, name="stats", tag="stats")
    nc.vector.tensor_copy(stats[0:64, :], psum_a[:])
    nc.vector.tensor_copy(stats[64:128, :], psum_b[:])
    nc.sync.dma_start(out=stats_d, in_=stats[:])

    ctx.close()


def _install_ntff_hook_shim():
    """This image's `antenv` lacks `axon_hooks`; replicate the boot-time
    NTFF profile hook (ctypes into libaxon_pjrt.so) so trace=True works."""
    try:
        from antenv.axon_hooks import get_axon_ntff_profile_hook  # noqa: F401

        return
    except ImportError:
        pass
    import contextlib
    import ctypes
    import types

    so_path = "/opt/axon/libaxon_pjrt.so"
    hook = None
    try:
        lib = ctypes.CDLL(so_path)
        if hasattr(lib, "axon_start_nrt_profile"):
            lib.axon_start_nrt_profile.argtypes = [
                ctypes.POINTER(ctypes.c_int64),
                ctypes.c_size_t,
            ]
            lib.axon_start_nrt_profile.restype = ctypes.c_int64
            lib.axon_stop_nrt_profile.argtypes = [ctypes.c_char_p]
            lib.axon_stop_nrt_profile.restype = ctypes.c_int64

            @contextlib.contextmanager
            def _hook(output_dir, device_ids):
                import jax

                jax.devices()
                if device_ids:
                    ids = (ctypes.c_int64 * len(device_ids))(*device_ids)
                    rc = lib.axon_start_nrt_profile(ids, len(device_ids))
                else:
                    rc = lib.axon_start_nrt_profile(None, 0)
                if rc != 0:
                    raise RuntimeError(f"axon_start_nrt_profile rc={rc}")
                try:
                    yield
                finally:
                    n = lib.axon_stop_nrt_profile(str(output_dir).encode())
                    if n < 0:
                        raise RuntimeError(f"axon_stop_nrt_profile rc={n}")
                    print(f"profile: {n} file(s) written to {output_dir}")

            hook = _hook
    except OSError:
        pass

    mod = types.ModuleType("antenv.axon_hooks")
    mod._hook = hook
    mod.get_axon_ntff_profile_hook = lambda: mod._hook
    mod.set_axon_ntff_profile_hook = lambda h: setattr(mod, "_hook", h)
    sys.modules["antenv.axon_hooks"] = mod


_CACHE = {}


def _get_compiled():
    if "nc" not in _CACHE:
        nc = bacc.Bacc(
            "TRN2", target_bir_lowering=False, debug=False, num_devices=NCORES
        )
        with tile.TileContext(nc) as tc:
            build_kernel(nc, tc)
        nc.compile()
        _CACHE["nc"] = nc
    return _CACHE["nc"]


def _wmat():
    bins = (np.arange(NBINS) * STEP + LOW).astype(np.float32)
    binc = np.zeros((2, 128), np.float32)
    binc[0] = bins[:128]
    binc[1, :127] = bins[128:]
    wm = np.zeros((72, 128, 32), np.float16)
    for t in range(T):
        for c in range(2):
            wm[2 * t + c, :, 2 * t % 32] = 1.0
            wm[2 * t + c, :, (2 * t + 1) % 32] = binc[c]
    for k in range(32):
        wm[32 + k, :, k] = 1.0
    for g in range(NG):
        for j in range(G):
            t = G * g + j
            wm[64 + g, 32 * j:32 * j + 32, t % 32] = 1.0
            wm[68 + g, 32 * j:32 * j + 32, (16 + t) % 32] = 1.0
    return np.ascontiguousarray(wm.transpose(1, 0, 2)).reshape(128, 72 * 32)


from ml_dtypes import float8_e4m3fn as _f8  # noqa: E402


def _stage_bins_tensor(x, dt):
    """[B, T, NBINS] fp32 -> per-core flat [NG, 128, G*2*R] so each
    SBUF partition line is one contiguous run: [g, p, (j c r)] =
    x[row=r, t=4g+j, bin=c*128+p]."""
    x8 = x.astype(dt)  # cast first: transpose then moves fewer bytes
    out = np.full((NCORES, T, NP2, R), dt(PAD_VAL), dt)
    # [core, row, t, n] -> [core, t, n, row]
    out[:, :, :NBINS, :] = x8.reshape(NCORES, R, T, NBINS).transpose(0, 2, 3, 1)
    # [core, (g j), (c p), r] -> [core, g, p, (j c r)]
    out = np.ascontiguousarray(
        out.reshape(NCORES, NG, G, 2, 128, R).transpose(0, 1, 4, 2, 3, 5)
    ).reshape(NCORES, NG, 128, G * 2 * R)
    return out


def _make_in_maps(inputs):
    rew = _stage_bins_tensor(np.asarray(inputs["predicted_reward_logits"]), _f8)
    slw = _stage_bins_tensor(np.asarray(inputs["slow_critic_logits"]), _f8)
    fst = _stage_bins_tensor(np.asarray(inputs["fast_critic_logits"]), _f8)
    actl = np.asarray(inputs["action_logits"]).astype(_f8)
    # [core, row, (g j), a] -> [core, g, (j a), row]
    actl_t = np.ascontiguousarray(
        actl.reshape(NCORES, R, NG, G, A).transpose(0, 2, 3, 4, 1)
    ).reshape(NCORES, NG, G * A, R)
    wm = _wmat()
    return [
        {
            "rew": rew[i],
            "slw": slw[i],
            "fst": fst[i],
            "actl": actl_t[i],
            "wmat": wm,
        }
        for i in range(NCORES)
    ]


def _combine(inputs, results):
    """Host tail in float64: decode stats, lambda scan, quantiles, two-hot
    gather, final scalar."""
    S = np.stack([np.asarray(r["stats"], dtype=np.float64) for r in results])
    # S: [core, 128, R]; reassemble [B, T] quantities (row = core*R + r)
    idx_t = np.arange(T)

    def grab(base, stride=1, off=0):
        # partitions base + stride*t (+off), -> [B, T]
        parts = S[:, base + off + stride * idx_t, :]  # [core, T, R]
        return parts.transpose(0, 2, 1).reshape(B, T)

    sum_r = grab(0, 2)
    wsum_r = grab(0, 2, 1)
    sum_s = grab(32, 2)
    wsum_s = grab(32, 2, 1)
    sum_f = grab(64, 2)
    fdot = grab(64, 2, 1)
    sum_a = grab(96)
    padot = grab(112)

    def symexp(y):
        return np.sign(y) * (np.exp(np.abs(y)) - 1.0)

    rewards = symexp(wsum_r / sum_r)
    values = symexp(wsum_s / sum_s)
    cont = np.asarray(
        inputs["predicted_continue_logits"], dtype=np.float64
    )[..., 0]
    continues = 1.0 / (1.0 + np.exp(-cont))

    # lambda returns (vectorized over B, reverse scan over T)
    lam_ret = np.empty((B, T), np.float64)
    lam_ret[:, -1] = values[:, -1]
    for t in range(T - 2, -1, -1):
        lam_ret[:, t] = rewards[:, t] + GAMMA * continues[:, t] * (
            (1.0 - LAM) * values[:, t + 1] + LAM * lam_ret[:, t + 1]
        )

    # ---- actor ----
    actl = np.asarray(inputs["action_logits"], dtype=np.float64)
    actions = np.asarray(inputs["actions"]).astype(np.int64)
    alp_raw = np.take_along_axis(actl, actions[..., None], axis=-1)[..., 0]
    lse_a = np.log(sum_a)
    alp = alp_raw - lse_a
    ent = lse_a - padot / sum_a
    flat = lam_ret.reshape(-1)
    p_hi = np.quantile(flat, 0.95)
    p_lo = np.quantile(flat, 0.05)
    norm = max(p_hi - p_lo, 1.0)
    norm_adv = (lam_ret - values) / norm
    actor = -np.mean(norm_adv * alp) - ENT_COEF * np.mean(ent)

    # ---- critic ----
    y2 = np.sign(lam_ret) * np.log1p(np.abs(lam_ret))
    pos = (np.clip(y2, LOW, HIGH) - LOW) / STEP
    k = np.clip(np.floor(pos), 0, NBINS - 2).astype(np.int64)
    w = pos - k
    fst = np.asarray(inputs["fast_critic_logits"], dtype=np.float64)
    fk = np.take_along_axis(fst, k[..., None], axis=-1)[..., 0]
    fk1 = np.take_along_axis(fst, k[..., None] + 1, axis=-1)[..., 0]
    g = (1.0 - w) * fk + w * fk1
    lse_f = np.log(sum_f) - LSE_F_BIAS
    fdn = fdot / sum_s
    critic = np.mean(lse_f - g) + SLOW_W * np.mean(lse_f - fdn)

    return np.float32(actor + critic)


def run(inputs, trace=False, **kw):
    if trace:
        _install_ntff_hook_shim()
    nc = _get_compiled()
    in_maps = _make_in_maps(inputs)
    res = bass_utils.run_bass_kernel_spmd(
        nc, in_maps, core_ids=list(range(NCORES)), trace=trace, **kw
    )
    return _combine(inputs, res.results), res


def kernel(**inputs) -> np.ndarray:
    out, _ = run(inputs)
    return out
